# revision 15
# baseline (speedup 1.0000x reference)
"""Trainium2 kernel for nn_AlignmentSimilarity.

Computation (per (b, n) pair):
    logits = (q_b - mean_q) @ s_n          # [Lq, Ls], C contracted
    P      = softmax(logits, axis=-1)      # exact, per-row max subtracted
    aligned_pair = P @ s_n^T - mean_s      # (softmax rows sum to 1 =>
                                           #  centering s reduces to -mean_s;
                                           #  in the logits it's softmax-invariant)
    aligned[b, g] = mean over the 5 shots n in group g
    q_out[b, g]   = q_b - mean_q

Sharding: core (2b + g) owns query b and shot group g (5 shots) -> the
shot average is core-local, no collectives. 8 cores cover B=4 x 2 groups.

Device pipeline per pair (layout: logits TRANSPOSED [j, i] so both matmuls
use natural channel-major operands and no big transposes are ever needed):
    MM1 (f32r, full PE rate): psum[j=128, i=1024] per j-chunk, 8 chunks.
    DMA copies psum -> SBUF f32 (idle DMA queues; frees PSUM).
    DVE pairwise-max tree over the 8 chunks -> [128, 1024].
    GPSIMD partition_all_reduce(max) -> rowmax m[i] broadcast to all
        partitions [128, 1024]   (the only cross-partition step).
    DVE in-place subtract: logits[jc] -= m  (aligned operands).
    ACT exp -> E bf16 in (0, 1]; overflow impossible, Z >= 1.
    MM2 (bf16): E^T-tile @ s^T_aug (ones column gives Z in psum col 256).
    DVE: recip(Z), acc += unnorm * rz (fused scalar_tensor_tensor).
Final: out = acc/5 - mean_s; host reassembles + transposes.
"""

import sys

sys.path.insert(0, "/opt/trn_rl_repo")

import ml_dtypes
import numpy as np

import concourse.bass as bass
import concourse.bass_isa as bass_isa
import concourse.mybir as mybir
import concourse.tile as tile
from concourse import bacc
from concourse.bass_utils import run_bass_kernel_spmd

F32 = mybir.dt.float32
F32R = mybir.dt.float32r
BF16 = mybir.dt.bfloat16
F16 = mybir.dt.float16

B, N, C, H, W = 4, 10, 256, 32, 32
L = H * W  # 1024
KSHOT = 5
NCORES = 8

last_exec_time_ns = None
last_result = None


def _build_graph(ms: float):
    nc = bacc.Bacc(
        "TRN2",
        target_bir_lowering=False,
        debug=False,
        num_devices=NCORES,
    )

    q_d = nc.declare_dram_parameter("q", [128, 2, L], F32R, isOutput=False)
    s_d = nc.declare_dram_parameter("s", [128, KSHOT, 2, L], F32R, isOutput=False)
    st_d = nc.declare_dram_parameter("st", [128, KSHOT, 8, C + 1], F16, isOutput=False)
    out_d = nc.declare_dram_parameter("out", [128, 8, C], F32, isOutput=True)

    MAX = mybir.AluOpType.max
    SUB = mybir.AluOpType.subtract

    with tile.TileContext(nc) as tc:
        with (
            tc.tile_pool(name="inp", bufs=1) as inp,
            tc.tile_pool(name="sp", bufs=3) as sp,
            tc.tile_pool(name="logp", bufs=3) as logp,
            tc.tile_pool(name="ep", bufs=3) as ep,
            tc.tile_pool(name="mp", bufs=2) as mp,
            tc.tile_pool(name="scr", bufs=2) as scr,
            tc.tile_pool(name="accp", bufs=1) as accp,
            tc.tile_pool(name="small", bufs=2) as small,
            tc.tile_pool(name="ps1", bufs=2, space=bass.MemorySpace.PSUM) as ps1p,
            tc.tile_pool(name="ps2", bufs=4, space=bass.MemorySpace.PSUM) as ps2p,
        ):
            q_sb = inp.tile([128, 2, L], F32R)
            nc.sync.dma_start(q_sb[:], q_d[:])

            acc = accp.tile([128, 8, C], F32)
            out_sb = accp.tile([128, 8, C], F32)

            tiles = {}

            def phase1(n):
                s_sb = sp.tile([128, 2, L], F32R, tag="S")
                nc.sync.dma_start(s_sb[:], s_d[:, n])
                st_sb = sp.tile([128, 8, C + 1], F16, tag="ST")
                nc.sync.dma_start(st_sb[:], st_d[:, n])
                log_sb = logp.tile([128, 8, L], F16, tag="LOG")
                e_sb = ep.tile([128, 8, L], F16, tag="E")
                mful = mp.tile([128, L], F16, tag="M")
                scratch = scr.tile([128, 4, L], F16, tag="SCR")
                tiles[n] = (st_sb, e_sb)

                # MM1' chunk loop with arrival-ordered max tree
                for jc in range(8):
                    ps1 = ps1p.tile([128, L], F32, tag="ps1")
                    for k in range(2):
                        lhsT = s_sb[:, k, jc * 128 : (jc + 1) * 128]
                        for ic in range(2):
                            rhs = q_sb[:, k, ic * 512 : (ic + 1) * 512]
                            nc.tensor.matmul(
                                ps1[:, ic * 512 : (ic + 1) * 512],
                                lhsT,
                                rhs,
                                start=(k == 0),
                                stop=(k == 1),
                            )
                    if jc in (3, 7):
                        nc.vector.tensor_copy(log_sb[:, jc, :], ps1[:])
                    else:
                        nc.scalar.activation(
                            log_sb[:, jc, :], ps1[:],
                            mybir.ActivationFunctionType.Copy,
                        )
                    # pairwise maxes as chunks arrive: slots 0..3 hold
                    # t01, t23, t45, t67; then fold into slot 0.
                    if jc % 2 == 1:
                        nc.vector.tensor_tensor(
                            scratch[:, jc // 2, :],
                            log_sb[:, jc - 1, :],
                            log_sb[:, jc, :],
                            MAX,
                        )
                    if jc == 3:
                        nc.vector.tensor_tensor(
                            scratch[:, 0, :], scratch[:, 0, :], scratch[:, 1, :], MAX
                        )
                    if jc == 7:
                        nc.vector.tensor_tensor(
                            scratch[:, 2, :], scratch[:, 2, :], scratch[:, 3, :], MAX
                        )
                        nc.vector.tensor_tensor(
                            scratch[:, 0, :], scratch[:, 0, :], scratch[:, 2, :], MAX
                        )
                nc.gpsimd.partition_all_reduce(
                    mful[:], scratch[:, 0, :], 128, bass_isa.ReduceOp.max
                )
                # x - rowmax (fp16 2x), exp -> E fp16 in (0, 1]
                for jc in range(8):
                    nc.vector.tensor_tensor(
                        log_sb[:, jc, :], log_sb[:, jc, :], mful[:], SUB
                    )
                    nc.scalar.activation(
                        e_sb[:, jc, :], log_sb[:, jc, :],
                        mybir.ActivationFunctionType.Exp,
                    )

            def phase2(n):
                st_sb, e_sb = tiles.pop(n)
                # two interleaved accumulation chains per it-pair: chain B's
                # LDWEIGHTS hides under chain A's matmul and vice versa.
                for itp in range(4):
                    psA = ps2p.tile([128, C + 1], F32, tag="ps2")
                    psB = ps2p.tile([128, C + 1], F32, tag="ps2")
                    itA, itB = 2 * itp, 2 * itp + 1
                    for jc in range(8):
                        nc.tensor.matmul(
                            psA[:],
                            e_sb[:, jc, itA * 128 : (itA + 1) * 128],
                            st_sb[:, jc, :],
                            start=(jc == 0),
                            stop=(jc == 7),
                        )
                        nc.tensor.matmul(
                            psB[:],
                            e_sb[:, jc, itB * 128 : (itB + 1) * 128],
                            st_sb[:, jc, :],
                            start=(jc == 0),
                            stop=(jc == 7),
                        )
                    for it, ps2 in ((itA, psA), (itB, psB)):
                        rc = small.tile([128, 1], F32, tag="rc")
                        nc.vector.reciprocal(rc[:], ps2[:, C : C + 1])
                        if n == 0:
                            nc.vector.tensor_scalar_mul(
                                acc[:, it, :], ps2[:, :C], rc[:]
                            )
                        else:
                            nc.vector.scalar_tensor_tensor(
                                acc[:, it, :],
                                ps2[:, :C],
                                rc[:],
                                acc[:, it, :],
                                mybir.AluOpType.mult,
                                mybir.AluOpType.add,
                            )

            LAG = 2
            for step in range(KSHOT + LAG):
                if step < KSHOT:
                    phase1(step)
                if step >= LAG:
                    phase2(step - LAG)

            # out = acc / KSHOT - mean_s
            nc.vector.tensor_scalar(
                out_sb[:],
                acc[:],
                1.0 / KSHOT,
                -float(ms),
                mybir.AluOpType.mult,
                mybir.AluOpType.add,
            )
            nc.sync.dma_start(out_d[:], out_sb[:])

    nc.compile()
    return nc


def kernel(query_features, support_features, K):
    global last_exec_time_ns, last_result
    q = np.asarray(query_features, dtype=np.float32).reshape(B, C, L)
    s = np.asarray(support_features, dtype=np.float32).reshape(N, C, L)
    assert int(K) == KSHOT

    mq = float(q.mean())
    ms = float(s.mean())
    qc = q - mq  # [B, C, L]

    # Per-core shards. Core 2b+g: query b, shots 5g..5g+4.
    in_maps = []
    for core in range(NCORES):
        b, g = core // 2, core % 2
        s5 = s[g * KSHOT : (g + 1) * KSHOT]  # [5, C, L]
        q_arr = np.ascontiguousarray(
            qc[b].reshape(2, 128, L).transpose(1, 0, 2)
        )  # [128, 2, L]
        s_arr = np.ascontiguousarray(
            s5.reshape(KSHOT, 2, 128, L).transpose(2, 0, 1, 3)
        )  # [128, 5, 2, L]
        st = np.empty((KSHOT, L, C + 1), dtype=np.float32)
        st[:, :, :C] = s5.transpose(0, 2, 1)
        st[:, :, C] = 1.0
        st_arr = np.ascontiguousarray(
            st.reshape(KSHOT, 8, 128, C + 1).transpose(2, 0, 1, 3)
        ).astype(np.float16)  # [128, 5, 8, 257]
        in_maps.append({"q": q_arr, "s": s_arr, "st": st_arr})

    nc = _build_graph(ms)
    res = run_bass_kernel_spmd(nc, in_maps, core_ids=list(range(NCORES)))
    last_exec_time_ns = res.exec_time_ns
    last_result = res

    # Gather: core output [128, 8, C] -> [L, C] (i = it*128 + p)
    aligned = np.empty((N // KSHOT, B, C, H, W), dtype=np.float32)
    for core in range(NCORES):
        b, g = core // 2, core % 2
        o = np.asarray(res.results[core]["out"])  # [128, 8, C]
        lc = o.transpose(1, 0, 2).reshape(L, C)  # [L, C]
        aligned[g, b] = lc.T.reshape(C, H, W)

    q_out = np.broadcast_to(
        qc.reshape(B, 1, C, H, W), (B, N // KSHOT, C, H, W)
    ).astype(np.float32)
    return q_out, aligned


# revision 16
# speedup vs baseline: 1.1396x; 1.1396x over previous
"""Trainium2 kernel for nn_AlignmentSimilarity.

Computation (per (b, n) pair):
    logits = (q_b - mean_q) @ s_n          # [Lq, Ls], C contracted
    P      = softmax(logits, axis=-1)      # exact, per-row max subtracted
    aligned_pair = P @ s_n^T - mean_s      # (softmax rows sum to 1 =>
                                           #  centering s reduces to -mean_s;
                                           #  in the logits it's softmax-invariant)
    aligned[b, g] = mean over the 5 shots n in group g
    q_out[b, g]   = q_b - mean_q

Sharding: core (2b + g) owns query b and shot group g (5 shots) -> the
shot average is core-local, no collectives. 8 cores cover B=4 x 2 groups.

Device pipeline per pair (layout: logits TRANSPOSED [j, i] so both matmuls
use natural channel-major operands and no big transposes are ever needed):
    MM1 (f32r, full PE rate): psum[j=128, i=1024] per j-chunk, 8 chunks.
    DMA copies psum -> SBUF f32 (idle DMA queues; frees PSUM).
    DVE pairwise-max tree over the 8 chunks -> [128, 1024].
    GPSIMD partition_all_reduce(max) -> rowmax m[i] broadcast to all
        partitions [128, 1024]   (the only cross-partition step).
    DVE in-place subtract: logits[jc] -= m  (aligned operands).
    ACT exp -> E bf16 in (0, 1]; overflow impossible, Z >= 1.
    MM2 (bf16): E^T-tile @ s^T_aug (ones column gives Z in psum col 256).
    DVE: recip(Z), acc += unnorm * rz (fused scalar_tensor_tensor).
Final: out = acc/5 - mean_s; host reassembles + transposes.
"""

import sys

sys.path.insert(0, "/opt/trn_rl_repo")

import ml_dtypes
import numpy as np

import concourse.bass as bass
import concourse.bass_isa as bass_isa
import concourse.mybir as mybir
import concourse.tile as tile
from concourse import bacc
from concourse.bass_utils import run_bass_kernel_spmd

F32 = mybir.dt.float32
F32R = mybir.dt.float32r
BF16 = mybir.dt.bfloat16
F16 = mybir.dt.float16

B, N, C, H, W = 4, 10, 256, 32, 32
L = H * W  # 1024
KSHOT = 5
NCORES = 8

last_exec_time_ns = None
last_result = None


def _build_graph(ms: float):
    nc = bacc.Bacc(
        "TRN2",
        target_bir_lowering=False,
        debug=False,
        num_devices=NCORES,
    )

    q_d = nc.declare_dram_parameter("q", [128, 2, L], F32R, isOutput=False)
    s_d = nc.declare_dram_parameter("s", [128, KSHOT, 2, L], F32R, isOutput=False)
    st_d = nc.declare_dram_parameter("st", [128, KSHOT, 8, C + 1], F16, isOutput=False)
    out_d = nc.declare_dram_parameter("out", [128, 8, C], F32, isOutput=True)

    MAX = mybir.AluOpType.max
    SUB = mybir.AluOpType.subtract

    with tile.TileContext(nc) as tc:
        with (
            tc.tile_pool(name="inp", bufs=1) as inp,
            tc.tile_pool(name="sp", bufs=3) as sp,
            tc.tile_pool(name="logp", bufs=3) as logp,
            tc.tile_pool(name="ep", bufs=3) as ep,
            tc.tile_pool(name="mp", bufs=2) as mp,
            tc.tile_pool(name="scr", bufs=2) as scr,
            tc.tile_pool(name="accp", bufs=1) as accp,
            tc.tile_pool(name="small", bufs=2) as small,
            tc.tile_pool(name="ps1", bufs=3, space=bass.MemorySpace.PSUM) as ps1p,
            tc.tile_pool(name="ps2", bufs=2, space=bass.MemorySpace.PSUM) as ps2p,
        ):
            q_sb = inp.tile([128, 2, L], F32R)
            nc.sync.dma_start(q_sb[:], q_d[:])

            acc = accp.tile([128, 8, C], F32)
            out_sb = accp.tile([128, 8, C], F32)

            tiles = {}

            def phase1(n):
                s_sb = sp.tile([128, 2, L], F32R, tag="S")
                nc.sync.dma_start(s_sb[:], s_d[:, n])
                st_sb = sp.tile([128, 8, C + 1], F16, tag="ST")
                nc.sync.dma_start(st_sb[:], st_d[:, n])
                log_sb = logp.tile([128, 8, L], F16, tag="LOG")
                e_sb = ep.tile([128, 8, L], F16, tag="E")
                mful = mp.tile([128, L], F16, tag="M")
                scratch = scr.tile([128, 4, L], F16, tag="SCR")
                tiles[n] = (st_sb, e_sb)

                # MM1' chunk loop with arrival-ordered max tree
                for jc in range(8):
                    ps1 = ps1p.tile([128, L], F32, tag="ps1")
                    for k in range(2):
                        lhsT = s_sb[:, k, jc * 128 : (jc + 1) * 128]
                        for ic in range(2):
                            rhs = q_sb[:, k, ic * 512 : (ic + 1) * 512]
                            nc.tensor.matmul(
                                ps1[:, ic * 512 : (ic + 1) * 512],
                                lhsT,
                                rhs,
                                start=(k == 0),
                                stop=(k == 1),
                            )
                    if jc in (3, 7):
                        nc.vector.tensor_copy(log_sb[:, jc, :], ps1[:])
                    else:
                        nc.scalar.activation(
                            log_sb[:, jc, :], ps1[:],
                            mybir.ActivationFunctionType.Copy,
                        )
                    # pairwise maxes as chunks arrive: slots 0..3 hold
                    # t01, t23, t45, t67; then fold into slot 0.
                    if jc % 2 == 1:
                        nc.vector.tensor_tensor(
                            scratch[:, jc // 2, :],
                            log_sb[:, jc - 1, :],
                            log_sb[:, jc, :],
                            MAX,
                        )
                    if jc == 3:
                        nc.vector.tensor_tensor(
                            scratch[:, 0, :], scratch[:, 0, :], scratch[:, 1, :], MAX
                        )
                    if jc == 7:
                        nc.vector.tensor_tensor(
                            scratch[:, 2, :], scratch[:, 2, :], scratch[:, 3, :], MAX
                        )
                        nc.vector.tensor_tensor(
                            scratch[:, 0, :], scratch[:, 0, :], scratch[:, 2, :], MAX
                        )
                nc.gpsimd.partition_all_reduce(
                    mful[:], scratch[:, 0, :], 128, bass_isa.ReduceOp.max
                )
                # x - rowmax (fp16 2x), exp -> E fp16 in (0, 1]
                for jc in range(8):
                    nc.vector.tensor_tensor(
                        log_sb[:, jc, :], log_sb[:, jc, :], mful[:], SUB
                    )
                    nc.scalar.activation(
                        e_sb[:, jc, :], log_sb[:, jc, :],
                        mybir.ActivationFunctionType.Exp,
                    )

            def phase2(n):
                st_sb, e_sb = tiles.pop(n)
                # two interleaved accumulation chains per it-pair: chain B's
                # LDWEIGHTS hides under chain A's matmul and vice versa.
                for itp in range(4):
                    psA = ps2p.tile([128, C + 1], F32, tag="ps2")
                    psB = ps2p.tile([128, C + 1], F32, tag="ps2")
                    itA, itB = 2 * itp, 2 * itp + 1
                    for jc in range(8):
                        nc.tensor.matmul(
                            psA[:],
                            e_sb[:, jc, itA * 128 : (itA + 1) * 128],
                            st_sb[:, jc, :],
                            start=(jc == 0),
                            stop=(jc == 7),
                        )
                        nc.tensor.matmul(
                            psB[:],
                            e_sb[:, jc, itB * 128 : (itB + 1) * 128],
                            st_sb[:, jc, :],
                            start=(jc == 0),
                            stop=(jc == 7),
                        )
                    for it, ps2 in ((itA, psA), (itB, psB)):
                        rc = small.tile([128, 1], F32, tag="rc")
                        nc.vector.reciprocal(rc[:], ps2[:, C : C + 1])
                        if n == 0:
                            nc.vector.tensor_scalar_mul(
                                acc[:, it, :], ps2[:, :C], rc[:]
                            )
                        else:
                            nc.vector.scalar_tensor_tensor(
                                acc[:, it, :],
                                ps2[:, :C],
                                rc[:],
                                acc[:, it, :],
                                mybir.AluOpType.mult,
                                mybir.AluOpType.add,
                            )

            LAG = 2
            for step in range(KSHOT + LAG):
                if step < KSHOT:
                    phase1(step)
                if step >= LAG:
                    phase2(step - LAG)

            # out = acc / KSHOT - mean_s
            nc.vector.tensor_scalar(
                out_sb[:],
                acc[:],
                1.0 / KSHOT,
                -float(ms),
                mybir.AluOpType.mult,
                mybir.AluOpType.add,
            )
            nc.sync.dma_start(out_d[:], out_sb[:])

    nc.compile()
    return nc


def kernel(query_features, support_features, K):
    global last_exec_time_ns, last_result
    q = np.asarray(query_features, dtype=np.float32).reshape(B, C, L)
    s = np.asarray(support_features, dtype=np.float32).reshape(N, C, L)
    assert int(K) == KSHOT

    mq = float(q.mean())
    ms = float(s.mean())
    qc = q - mq  # [B, C, L]

    # Per-core shards. Core 2b+g: query b, shots 5g..5g+4.
    in_maps = []
    for core in range(NCORES):
        b, g = core // 2, core % 2
        s5 = s[g * KSHOT : (g + 1) * KSHOT]  # [5, C, L]
        q_arr = np.ascontiguousarray(
            qc[b].reshape(2, 128, L).transpose(1, 0, 2)
        )  # [128, 2, L]
        s_arr = np.ascontiguousarray(
            s5.reshape(KSHOT, 2, 128, L).transpose(2, 0, 1, 3)
        )  # [128, 5, 2, L]
        st = np.empty((KSHOT, L, C + 1), dtype=np.float32)
        st[:, :, :C] = s5.transpose(0, 2, 1)
        st[:, :, C] = 1.0
        st_arr = np.ascontiguousarray(
            st.reshape(KSHOT, 8, 128, C + 1).transpose(2, 0, 1, 3)
        ).astype(np.float16)  # [128, 5, 8, 257]
        in_maps.append({"q": q_arr, "s": s_arr, "st": st_arr})

    nc = _build_graph(ms)
    res = run_bass_kernel_spmd(nc, in_maps, core_ids=list(range(NCORES)))
    last_exec_time_ns = res.exec_time_ns
    last_result = res

    # Gather: core output [128, 8, C] -> [L, C] (i = it*128 + p)
    aligned = np.empty((N // KSHOT, B, C, H, W), dtype=np.float32)
    for core in range(NCORES):
        b, g = core // 2, core % 2
        o = np.asarray(res.results[core]["out"])  # [128, 8, C]
        lc = o.transpose(1, 0, 2).reshape(L, C)  # [L, C]
        aligned[g, b] = lc.T.reshape(C, H, W)

    q_out = np.broadcast_to(
        qc.reshape(B, 1, C, H, W), (B, N // KSHOT, C, H, W)
    ).astype(np.float32)
    return q_out, aligned


# revision 17
# speedup vs baseline: 1.2365x; 1.0850x over previous
"""Trainium2 kernel for nn_AlignmentSimilarity.

Computation (per (b, n) pair):
    logits = (q_b - mean_q) @ s_n          # [Lq, Ls], C contracted
    P      = softmax(logits, axis=-1)      # exact, per-row max subtracted
    aligned_pair = P @ s_n^T - mean_s      # (softmax rows sum to 1 =>
                                           #  centering s reduces to -mean_s;
                                           #  in the logits it's softmax-invariant)
    aligned[b, g] = mean over the 5 shots n in group g
    q_out[b, g]   = q_b - mean_q

Sharding: core (2b + g) owns query b and shot group g (5 shots) -> the
shot average is core-local, no collectives. 8 cores cover B=4 x 2 groups.

Device pipeline per pair (layout: logits TRANSPOSED [j, i] so both matmuls
use natural channel-major operands and no big transposes are ever needed):
    MM1 (f32r, full PE rate): psum[j=128, i=1024] per j-chunk, 8 chunks.
    DMA copies psum -> SBUF f32 (idle DMA queues; frees PSUM).
    DVE pairwise-max tree over the 8 chunks -> [128, 1024].
    GPSIMD partition_all_reduce(max) -> rowmax m[i] broadcast to all
        partitions [128, 1024]   (the only cross-partition step).
    DVE in-place subtract: logits[jc] -= m  (aligned operands).
    ACT exp -> E bf16 in (0, 1]; overflow impossible, Z >= 1.
    MM2 (bf16): E^T-tile @ s^T_aug (ones column gives Z in psum col 256).
    DVE: recip(Z), acc += unnorm * rz (fused scalar_tensor_tensor).
Final: out = acc/5 - mean_s; host reassembles + transposes.
"""

import sys

sys.path.insert(0, "/opt/trn_rl_repo")

import ml_dtypes
import numpy as np

import concourse.bass as bass
import concourse.bass_isa as bass_isa
import concourse.mybir as mybir
import concourse.tile as tile
from concourse import bacc
from concourse.bass_utils import run_bass_kernel_spmd

F32 = mybir.dt.float32
F32R = mybir.dt.float32r
BF16 = mybir.dt.bfloat16
F16 = mybir.dt.float16

B, N, C, H, W = 4, 10, 256, 32, 32
L = H * W  # 1024
KSHOT = 5
NCORES = 8

last_exec_time_ns = None
last_result = None


def _build_graph(ms: float):
    nc = bacc.Bacc(
        "TRN2",
        target_bir_lowering=False,
        debug=False,
        num_devices=NCORES,
    )

    q_d = nc.declare_dram_parameter("q", [128, 2, L], F16, isOutput=False)
    s_d = nc.declare_dram_parameter("s", [128, KSHOT, 2, L], F16, isOutput=False)
    st_d = nc.declare_dram_parameter("st", [128, KSHOT, 8, C + 1], F16, isOutput=False)
    out_d = nc.declare_dram_parameter("out", [128, 8, C], F32, isOutput=True)

    MAX = mybir.AluOpType.max
    SUB = mybir.AluOpType.subtract

    with tile.TileContext(nc) as tc:
        with (
            tc.tile_pool(name="inp", bufs=1) as inp,
            tc.tile_pool(name="sp", bufs=3) as sp,
            tc.tile_pool(name="logp", bufs=3) as logp,
            tc.tile_pool(name="ep", bufs=3) as ep,
            tc.tile_pool(name="mp", bufs=2) as mp,
            tc.tile_pool(name="scr", bufs=2) as scr,
            tc.tile_pool(name="accp", bufs=1) as accp,
            tc.tile_pool(name="small", bufs=2) as small,
            tc.tile_pool(name="ps1", bufs=3, space=bass.MemorySpace.PSUM) as ps1p,
            tc.tile_pool(name="ps2", bufs=2, space=bass.MemorySpace.PSUM) as ps2p,
        ):
            q_sb = inp.tile([128, 2, L], F16)
            nc.sync.dma_start(q_sb[:], q_d[:])

            acc = accp.tile([128, 8, C], F32)
            out_sb = accp.tile([128, 8, C], F32)

            tiles = {}

            def phase1(n):
                s_sb = sp.tile([128, 2, L], F16, tag="S")
                nc.sync.dma_start(s_sb[:], s_d[:, n])
                st_sb = sp.tile([128, 8, C + 1], F16, tag="ST")
                nc.sync.dma_start(st_sb[:], st_d[:, n])
                log_sb = logp.tile([128, 8, L], F16, tag="LOG")
                e_sb = ep.tile([128, 8, L], F16, tag="E")
                mful = mp.tile([128, L], F16, tag="M")
                scratch = scr.tile([128, 4, L], F16, tag="SCR")
                tiles[n] = (st_sb, e_sb)

                # MM1' chunk loop with arrival-ordered max tree
                for jc in range(8):
                    ps1 = ps1p.tile([128, L], F32, tag="ps1")
                    for k in range(2):
                        lhsT = s_sb[:, k, jc * 128 : (jc + 1) * 128]
                        for ic in range(2):
                            rhs = q_sb[:, k, ic * 512 : (ic + 1) * 512]
                            nc.tensor.matmul(
                                ps1[:, ic * 512 : (ic + 1) * 512],
                                lhsT,
                                rhs,
                                start=(k == 0),
                                stop=(k == 1),
                            )
                    if jc in (3, 7):
                        nc.vector.tensor_copy(log_sb[:, jc, :], ps1[:])
                    else:
                        nc.scalar.activation(
                            log_sb[:, jc, :], ps1[:],
                            mybir.ActivationFunctionType.Copy,
                        )
                    # pairwise maxes as chunks arrive: slots 0..3 hold
                    # t01, t23, t45, t67; then fold into slot 0.
                    if jc % 2 == 1:
                        nc.vector.tensor_tensor(
                            scratch[:, jc // 2, :],
                            log_sb[:, jc - 1, :],
                            log_sb[:, jc, :],
                            MAX,
                        )
                    if jc == 3:
                        nc.vector.tensor_tensor(
                            scratch[:, 0, :], scratch[:, 0, :], scratch[:, 1, :], MAX
                        )
                    if jc == 7:
                        nc.vector.tensor_tensor(
                            scratch[:, 2, :], scratch[:, 2, :], scratch[:, 3, :], MAX
                        )
                        nc.vector.tensor_tensor(
                            scratch[:, 0, :], scratch[:, 0, :], scratch[:, 2, :], MAX
                        )
                nc.gpsimd.partition_all_reduce(
                    mful[:], scratch[:, 0, :], 128, bass_isa.ReduceOp.max
                )
                # x - rowmax (fp16 2x), exp -> E fp16 in (0, 1]
                for jc in range(8):
                    nc.vector.tensor_tensor(
                        log_sb[:, jc, :], log_sb[:, jc, :], mful[:], SUB
                    )
                    nc.scalar.activation(
                        e_sb[:, jc, :], log_sb[:, jc, :],
                        mybir.ActivationFunctionType.Exp,
                    )

            def phase2(n):
                st_sb, e_sb = tiles.pop(n)
                for it in range(8):
                    ps2 = ps2p.tile([128, C + 1], F32, tag="ps2")
                    for jc in range(8):
                        nc.tensor.matmul(
                            ps2[:],
                            e_sb[:, jc, it * 128 : (it + 1) * 128],
                            st_sb[:, jc, :],
                            start=(jc == 0),
                            stop=(jc == 7),
                        )
                    rc = small.tile([128, 1], F32, tag="rc")
                    nc.vector.reciprocal(rc[:], ps2[:, C : C + 1])
                    if n == 0:
                        nc.vector.tensor_scalar_mul(acc[:, it, :], ps2[:, :C], rc[:])
                    else:
                        nc.vector.scalar_tensor_tensor(
                            acc[:, it, :],
                            ps2[:, :C],
                            rc[:],
                            acc[:, it, :],
                            mybir.AluOpType.mult,
                            mybir.AluOpType.add,
                        )

            LAG = 2
            for step in range(KSHOT + LAG):
                if step < KSHOT:
                    phase1(step)
                if step >= LAG:
                    phase2(step - LAG)

            # out = acc / KSHOT - mean_s
            nc.vector.tensor_scalar(
                out_sb[:],
                acc[:],
                1.0 / KSHOT,
                -float(ms),
                mybir.AluOpType.mult,
                mybir.AluOpType.add,
            )
            nc.sync.dma_start(out_d[:], out_sb[:])

    nc.compile()
    return nc


def kernel(query_features, support_features, K):
    global last_exec_time_ns, last_result
    q = np.asarray(query_features, dtype=np.float32).reshape(B, C, L)
    s = np.asarray(support_features, dtype=np.float32).reshape(N, C, L)
    assert int(K) == KSHOT

    mq = float(q.mean())
    ms = float(s.mean())
    qc = q - mq  # [B, C, L]

    # Per-core shards. Core 2b+g: query b, shots 5g..5g+4.
    in_maps = []
    for core in range(NCORES):
        b, g = core // 2, core % 2
        s5 = s[g * KSHOT : (g + 1) * KSHOT]  # [5, C, L]
        q_arr = np.ascontiguousarray(
            qc[b].reshape(2, 128, L).transpose(1, 0, 2)
        ).astype(np.float16)  # [128, 2, L]
        s_arr = np.ascontiguousarray(
            s5.reshape(KSHOT, 2, 128, L).transpose(2, 0, 1, 3)
        ).astype(np.float16)  # [128, 5, 2, L]
        st = np.empty((KSHOT, L, C + 1), dtype=np.float32)
        st[:, :, :C] = s5.transpose(0, 2, 1)
        st[:, :, C] = 1.0
        st_arr = np.ascontiguousarray(
            st.reshape(KSHOT, 8, 128, C + 1).transpose(2, 0, 1, 3)
        ).astype(np.float16)  # [128, 5, 8, 257]
        in_maps.append({"q": q_arr, "s": s_arr, "st": st_arr})

    nc = _build_graph(ms)
    res = run_bass_kernel_spmd(nc, in_maps, core_ids=list(range(NCORES)))
    last_exec_time_ns = res.exec_time_ns
    last_result = res

    # Gather: core output [128, 8, C] -> [L, C] (i = it*128 + p)
    aligned = np.empty((N // KSHOT, B, C, H, W), dtype=np.float32)
    for core in range(NCORES):
        b, g = core // 2, core % 2
        o = np.asarray(res.results[core]["out"])  # [128, 8, C]
        lc = o.transpose(1, 0, 2).reshape(L, C)  # [L, C]
        aligned[g, b] = lc.T.reshape(C, H, W)

    q_out = np.broadcast_to(
        qc.reshape(B, 1, C, H, W), (B, N // KSHOT, C, H, W)
    ).astype(np.float32)
    return q_out, aligned


# revision 18
# speedup vs baseline: 1.2471x; 1.0086x over previous
"""Trainium2 kernel for nn_AlignmentSimilarity.

Computation (per (b, n) pair):
    logits = (q_b - mean_q) @ s_n          # [Lq, Ls], C contracted
    P      = softmax(logits, axis=-1)      # exact, per-row max subtracted
    aligned_pair = P @ s_n^T - mean_s      # (softmax rows sum to 1 =>
                                           #  centering s reduces to -mean_s;
                                           #  in the logits it's softmax-invariant)
    aligned[b, g] = mean over the 5 shots n in group g
    q_out[b, g]   = q_b - mean_q

Sharding: core (2b + g) owns query b and shot group g (5 shots) -> the
shot average is core-local, no collectives. 8 cores cover B=4 x 2 groups.

Device pipeline per pair (layout: logits TRANSPOSED [j, i] so both matmuls
use natural channel-major operands and no big transposes are ever needed):
    MM1 (f32r, full PE rate): psum[j=128, i=1024] per j-chunk, 8 chunks.
    DMA copies psum -> SBUF f32 (idle DMA queues; frees PSUM).
    DVE pairwise-max tree over the 8 chunks -> [128, 1024].
    GPSIMD partition_all_reduce(max) -> rowmax m[i] broadcast to all
        partitions [128, 1024]   (the only cross-partition step).
    DVE in-place subtract: logits[jc] -= m  (aligned operands).
    ACT exp -> E bf16 in (0, 1]; overflow impossible, Z >= 1.
    MM2 (bf16): E^T-tile @ s^T_aug (ones column gives Z in psum col 256).
    DVE: recip(Z), acc += unnorm * rz (fused scalar_tensor_tensor).
Final: out = acc/5 - mean_s; host reassembles + transposes.
"""

import sys

sys.path.insert(0, "/opt/trn_rl_repo")

import ml_dtypes
import numpy as np

import concourse.bass as bass
import concourse.bass_isa as bass_isa
import concourse.mybir as mybir
import concourse.tile as tile
from concourse import bacc
from concourse.bass_utils import run_bass_kernel_spmd

F32 = mybir.dt.float32
F32R = mybir.dt.float32r
BF16 = mybir.dt.bfloat16
F16 = mybir.dt.float16

B, N, C, H, W = 4, 10, 256, 32, 32
L = H * W  # 1024
KSHOT = 5
NCORES = 8

last_exec_time_ns = None
last_result = None


def _build_graph(ms: float):
    nc = bacc.Bacc(
        "TRN2",
        target_bir_lowering=False,
        debug=False,
        num_devices=NCORES,
    )

    q_d = nc.declare_dram_parameter("q", [128, 2, L], F16, isOutput=False)
    s_d = nc.declare_dram_parameter("s", [128, KSHOT, 2, L], F16, isOutput=False)
    st_d = nc.declare_dram_parameter("st", [128, KSHOT, 8, C + 1], F16, isOutput=False)
    out_d = nc.declare_dram_parameter("out", [128, 8, C], F32, isOutput=True)

    MAX = mybir.AluOpType.max
    SUB = mybir.AluOpType.subtract

    with tile.TileContext(nc) as tc:
        with (
            tc.tile_pool(name="inp", bufs=1) as inp,
            tc.tile_pool(name="sp", bufs=2) as sp,
            tc.tile_pool(name="stp", bufs=5) as stp,
            tc.tile_pool(name="logp", bufs=2) as logp,
            tc.tile_pool(name="ep", bufs=4) as ep,
            tc.tile_pool(name="mp", bufs=2) as mp,
            tc.tile_pool(name="scr", bufs=2) as scr,
            tc.tile_pool(name="accp", bufs=1) as accp,
            tc.tile_pool(name="small", bufs=2) as small,
            tc.tile_pool(name="ps1", bufs=3, space=bass.MemorySpace.PSUM) as ps1p,
            tc.tile_pool(name="ps2", bufs=2, space=bass.MemorySpace.PSUM) as ps2p,
        ):
            q_sb = inp.tile([128, 2, L], F16)
            nc.sync.dma_start(q_sb[:], q_d[:])

            acc = accp.tile([128, 8, C], F32)
            out_sb = accp.tile([128, 8, C], F32)

            tiles = {}

            def phase1(n):
                s_sb = sp.tile([128, 2, L], F16, tag="S")
                nc.sync.dma_start(s_sb[:], s_d[:, n])
                st_sb = stp.tile([128, 8, C + 1], F16, tag="ST")
                nc.sync.dma_start(st_sb[:], st_d[:, n])
                log_sb = logp.tile([128, 8, L], F16, tag="LOG")
                e_sb = ep.tile([128, 8, L], F16, tag="E")
                mful = mp.tile([128, L], F16, tag="M")
                scratch = scr.tile([128, 4, L], F16, tag="SCR")
                tiles[n] = (st_sb, e_sb)

                # MM1' chunk loop with arrival-ordered max tree
                for jc in range(8):
                    ps1 = ps1p.tile([128, L], F32, tag="ps1")
                    for k in range(2):
                        lhsT = s_sb[:, k, jc * 128 : (jc + 1) * 128]
                        for ic in range(2):
                            rhs = q_sb[:, k, ic * 512 : (ic + 1) * 512]
                            nc.tensor.matmul(
                                ps1[:, ic * 512 : (ic + 1) * 512],
                                lhsT,
                                rhs,
                                start=(k == 0),
                                stop=(k == 1),
                            )
                    if jc in (3, 7):
                        nc.vector.tensor_copy(log_sb[:, jc, :], ps1[:])
                    else:
                        nc.scalar.activation(
                            log_sb[:, jc, :], ps1[:],
                            mybir.ActivationFunctionType.Copy,
                        )
                    # pairwise maxes as chunks arrive: slots 0..3 hold
                    # t01, t23, t45, t67; then fold into slot 0.
                    if jc % 2 == 1:
                        nc.vector.tensor_tensor(
                            scratch[:, jc // 2, :],
                            log_sb[:, jc - 1, :],
                            log_sb[:, jc, :],
                            MAX,
                        )
                    if jc == 3:
                        nc.vector.tensor_tensor(
                            scratch[:, 0, :], scratch[:, 0, :], scratch[:, 1, :], MAX
                        )
                    if jc == 7:
                        nc.vector.tensor_tensor(
                            scratch[:, 2, :], scratch[:, 2, :], scratch[:, 3, :], MAX
                        )
                        nc.vector.tensor_tensor(
                            scratch[:, 0, :], scratch[:, 0, :], scratch[:, 2, :], MAX
                        )
                nc.gpsimd.partition_all_reduce(
                    mful[:], scratch[:, 0, :], 128, bass_isa.ReduceOp.max
                )
                # x - rowmax (fp16 2x), exp -> E fp16 in (0, 1]
                for jc in range(8):
                    nc.vector.tensor_tensor(
                        log_sb[:, jc, :], log_sb[:, jc, :], mful[:], SUB
                    )
                    if jc % 2 == 1:
                        nc.scalar.activation(
                            e_sb[:, jc - 1 : jc + 1, :],
                            log_sb[:, jc - 1 : jc + 1, :],
                            mybir.ActivationFunctionType.Exp,
                        )

            def phase2(n):
                st_sb, e_sb = tiles.pop(n)
                for it in range(8):
                    ps2 = ps2p.tile([128, C + 1], F32, tag="ps2")
                    for jc in range(8):
                        nc.tensor.matmul(
                            ps2[:],
                            e_sb[:, jc, it * 128 : (it + 1) * 128],
                            st_sb[:, jc, :],
                            start=(jc == 0),
                            stop=(jc == 7),
                        )
                    rc = small.tile([128, 1], F32, tag="rc")
                    nc.vector.reciprocal(rc[:], ps2[:, C : C + 1])
                    if n == 0:
                        nc.vector.tensor_scalar_mul(acc[:, it, :], ps2[:, :C], rc[:])
                    else:
                        nc.vector.scalar_tensor_tensor(
                            acc[:, it, :],
                            ps2[:, :C],
                            rc[:],
                            acc[:, it, :],
                            mybir.AluOpType.mult,
                            mybir.AluOpType.add,
                        )

            LAG = 3
            for step in range(KSHOT + LAG):
                if step < KSHOT:
                    phase1(step)
                if step >= LAG:
                    phase2(step - LAG)

            # out = acc / KSHOT - mean_s
            nc.vector.tensor_scalar(
                out_sb[:],
                acc[:],
                1.0 / KSHOT,
                -float(ms),
                mybir.AluOpType.mult,
                mybir.AluOpType.add,
            )
            nc.sync.dma_start(out_d[:], out_sb[:])

    nc.compile()
    return nc


def kernel(query_features, support_features, K):
    global last_exec_time_ns, last_result
    q = np.asarray(query_features, dtype=np.float32).reshape(B, C, L)
    s = np.asarray(support_features, dtype=np.float32).reshape(N, C, L)
    assert int(K) == KSHOT

    mq = float(q.mean())
    ms = float(s.mean())
    qc = q - mq  # [B, C, L]

    # Per-core shards. Core 2b+g: query b, shots 5g..5g+4.
    in_maps = []
    for core in range(NCORES):
        b, g = core // 2, core % 2
        s5 = s[g * KSHOT : (g + 1) * KSHOT]  # [5, C, L]
        q_arr = np.ascontiguousarray(
            qc[b].reshape(2, 128, L).transpose(1, 0, 2)
        ).astype(np.float16)  # [128, 2, L]
        s_arr = np.ascontiguousarray(
            s5.reshape(KSHOT, 2, 128, L).transpose(2, 0, 1, 3)
        ).astype(np.float16)  # [128, 5, 2, L]
        st = np.empty((KSHOT, L, C + 1), dtype=np.float32)
        st[:, :, :C] = s5.transpose(0, 2, 1)
        st[:, :, C] = 1.0
        st_arr = np.ascontiguousarray(
            st.reshape(KSHOT, 8, 128, C + 1).transpose(2, 0, 1, 3)
        ).astype(np.float16)  # [128, 5, 8, 257]
        in_maps.append({"q": q_arr, "s": s_arr, "st": st_arr})

    nc = _build_graph(ms)
    res = run_bass_kernel_spmd(nc, in_maps, core_ids=list(range(NCORES)))
    last_exec_time_ns = res.exec_time_ns
    last_result = res

    # Gather: core output [128, 8, C] -> [L, C] (i = it*128 + p)
    aligned = np.empty((N // KSHOT, B, C, H, W), dtype=np.float32)
    for core in range(NCORES):
        b, g = core // 2, core % 2
        o = np.asarray(res.results[core]["out"])  # [128, 8, C]
        lc = o.transpose(1, 0, 2).reshape(L, C)  # [L, C]
        aligned[g, b] = lc.T.reshape(C, H, W)

    q_out = np.broadcast_to(
        qc.reshape(B, 1, C, H, W), (B, N // KSHOT, C, H, W)
    ).astype(np.float32)
    return q_out, aligned
